# revision 1
# baseline (speedup 1.0000x reference)
"""Trainium2 Bass kernel: e3nn-style GNN convolution (FC-net edge weights ->
FullyConnectedTensorProduct -> scatter-sum over edge_dst).

Strategy (edge-parallel, dst-sharded):
  * Sort edges by dst on host. Core c owns dst nodes [2500c, 2500(c+1)).
  * Per core: 20 "blocks" of 128 output nodes. Each block's edges are padded
    to a fixed number of 128-edge tiles (t_b, computed from data) so all 8
    cores run one identical SPMD program.
  * Per 128-edge tile (edges live on SBUF partitions):
      - indirect-DMA gather of node_features[src]  -> x [128,64]
      - PE: h^T = relu(fc_w1^T @ sc^T)  [256,128]; w = h^T.T @ fc_w2p [128,1024]
      - DVE: per-edge tensor-product contractions (paths 1..4) -> feat [128,64]
      - PE: selection-matrix (dst one-hot) matmul accumulates the block's
        [128,64] output in PSUM across the block's tiles.
  * Block PSUM -> DRAM; host concatenates the 8 node-sharded outputs.

All normalization constants (1/sqrt(3) fc layer, 1/sqrt(256) fc layer,
1/sqrt(3) dot, 1/sqrt(2*MUL) path norm, 1/sqrt(16) neighbor norm) are folded
into fc_w1/fc_w2 on the host.
"""

import math

import numpy as np

N_NODES = 20000
N_CORES = 8
NODES_PER_CORE = N_NODES // N_CORES  # 2500
BLK = 128
BLOCKS = (NODES_PER_CORE + BLK - 1) // BLK  # 20
P = 128
MUL = 16

_CACHE: dict = {}


def _build(n_tiles: int, t_b: int, blocks: int = BLOCKS, n_nodes: int = N_NODES):
    import concourse.bass as bass
    import concourse.mybir as mybir
    import concourse.tile as tile
    from concourse import bacc

    dt = mybir.dt
    Alu = mybir.AluOpType
    Act = mybir.ActivationFunctionType

    nc = bacc.Bacc("TRN2", target_bir_lowering=False, debug=False)

    xg_tab = nc.dram_tensor("xg_tab", [n_nodes, 64], dt.float32, kind="ExternalInput")
    srcT = nc.dram_tensor("srcT", [P, n_tiles], dt.int32, kind="ExternalInput")
    dstf = nc.dram_tensor("dstf", [P, n_tiles], dt.float32, kind="ExternalInput")
    attrP = nc.dram_tensor("attrP", [P, 4 * n_tiles], dt.float32, kind="ExternalInput")
    scT = nc.dram_tensor("scT", [3, n_tiles * P], dt.float32, kind="ExternalInput")
    fw1 = nc.dram_tensor("fw1", [3, 256], dt.float32, kind="ExternalInput")
    fw2 = nc.dram_tensor("fw2", [256, 1024], dt.float32, kind="ExternalInput")
    outp = nc.dram_tensor("outp", [blocks * BLK, 64], dt.float32, kind="ExternalOutput")

    with tile.TileContext(nc) as tc:
        with (
            tc.tile_pool(name="const", bufs=1) as cp,
            tc.tile_pool(name="sb", bufs=3) as sb,
            tc.tile_pool(name="scp", bufs=2) as scp,
            tc.tile_pool(name="wps", bufs=2, space="PSUM") as wps,
            tc.tile_pool(name="hps", bufs=2, space="PSUM") as hps,
            tc.tile_pool(name="aps", bufs=2, space="PSUM") as aps,
        ):
            iota_i = cp.tile([P, P], dt.int32)
            nc.gpsimd.iota(iota_i[:], pattern=[[1, P]], base=0, channel_multiplier=0)
            iota_f = cp.tile([P, P], dt.float32)
            nc.vector.tensor_copy(iota_f[:], iota_i[:])

            srcT_sb = cp.tile([P, n_tiles], dt.int32)
            nc.sync.dma_start(srcT_sb[:], srcT[:])
            dstf_sb = cp.tile([P, n_tiles], dt.float32)
            nc.sync.dma_start(dstf_sb[:], dstf[:])
            attr_sb = cp.tile([P, 4 * n_tiles], dt.float32)
            nc.sync.dma_start(attr_sb[:], attrP[:])
            fw1_sb = cp.tile([3, 256], dt.float32)
            nc.sync.dma_start(fw1_sb[:], fw1[:])
            fw2_sb = cp.tile([P, 2048], dt.float32)
            nc.sync.dma_start(fw2_sb[:, 0:1024], fw2[0:128, :])
            nc.sync.dma_start(fw2_sb[:, 1024:2048], fw2[128:256, :])

            for b in range(blocks):
                acc = aps.tile([P, 64], dt.float32, tag="acc")
                scc = scp.tile([3, t_b * P], dt.float32, tag="scc")
                nc.sync.dma_start(scc[:], scT[:, b * t_b * P : (b + 1) * t_b * P])
                for j in range(t_b):
                    t = b * t_b + j
                    xg = sb.tile([P, 64], dt.float32, tag="xg")
                    nc.gpsimd.indirect_dma_start(
                        out=xg[:],
                        out_offset=None,
                        in_=xg_tab[:],
                        in_offset=bass.IndirectOffsetOnAxis(
                            ap=srcT_sb[:, t : t + 1], axis=0
                        ),
                    )
                    # FC net: h^T [k, e] in PSUM, two k-chunks side by side
                    hp = hps.tile([P, 256], dt.float32, tag="hp")
                    rhs_sc = scc[:, j * P : (j + 1) * P]
                    nc.tensor.matmul(
                        out=hp[:, 0:128], lhsT=fw1_sb[:, 0:128], rhs=rhs_sc,
                        start=True, stop=True,
                    )
                    nc.tensor.matmul(
                        out=hp[:, 128:256], lhsT=fw1_sb[:, 128:256], rhs=rhs_sc,
                        start=True, stop=True,
                    )
                    h_sb = sb.tile([P, 256], dt.float32, tag="h")
                    nc.scalar.activation(h_sb[:], hp[:], Act.Relu)
                    # per-edge weights w [e, (path,o,i)] in PSUM
                    wp = wps.tile([P, 1024], dt.float32, tag="wp")
                    for nh in range(2):
                        for kc in range(2):
                            nc.tensor.matmul(
                                out=wp[:, nh * 512 : (nh + 1) * 512],
                                lhsT=h_sb[:, kc * 128 : (kc + 1) * 128],
                                rhs=fw2_sb[
                                    :, kc * 1024 + nh * 512 : kc * 1024 + (nh + 1) * 512
                                ],
                                start=(kc == 0),
                                stop=(kc == 1),
                            )
                    # FCTP (DVE), edges on partitions
                    shs = attr_sb[:, 4 * t : 4 * t + 1]
                    shv = attr_sb[:, 4 * t + 1 : 4 * t + 4]
                    s_ap = xg[:, 0:16]
                    v_ic = xg[:, 16:64].rearrange("p (i c) -> p i c", c=3)
                    v_ci = xg[:, 16:64].rearrange("p (i c) -> p c i", c=3)

                    alpha = sb.tile([P, 32], dt.float32, tag="alpha")
                    nc.vector.tensor_copy(alpha[:, 0:16], s_ap)
                    tmp_d = sb.tile([P, 48], dt.float32, tag="tmpd")
                    nc.vector.tensor_tensor(
                        out=tmp_d[:].rearrange("p (i c) -> p i c", c=3),
                        in0=v_ic,
                        in1=shv.unsqueeze(1).broadcast_to([P, 16, 3]),
                        op=Alu.mult,
                    )
                    nc.vector.tensor_reduce(
                        out=alpha[:, 16:32],
                        in_=tmp_d[:].rearrange("p (i c) -> p i c", c=3),
                        axis=mybir.AxisListType.X,
                        op=Alu.add,
                    )

                    tmp13 = sb.tile([P, 768], dt.float32, tag="tmp13")
                    nc.vector.tensor_tensor(
                        out=tmp13[:, 0:512].rearrange("p (a o i) -> p a o i", a=2, o=16),
                        in0=wp[:, 0:512].rearrange("p (a o i) -> p a o i", a=2, o=16),
                        in1=alpha[:]
                        .rearrange("p (a i) -> p a i", a=2)
                        .unsqueeze(2)
                        .broadcast_to([P, 2, 16, 16]),
                        op=Alu.mult,
                    )
                    nc.vector.tensor_tensor(
                        out=tmp13[:, 512:768].rearrange("p (o i) -> p o i", o=16),
                        in0=wp[:, 512:768].rearrange("p (o i) -> p o i", o=16),
                        in1=s_ap.unsqueeze(1).broadcast_to([P, 16, 16]),
                        op=Alu.mult,
                    )
                    M = sb.tile([P, 48], dt.float32, tag="M")
                    nc.vector.tensor_reduce(
                        out=M[:],
                        in_=tmp13[:].rearrange("p (g i) -> p g i", i=16),
                        axis=mybir.AxisListType.X,
                        op=Alu.add,
                    )

                    tmp4 = sb.tile([P, 768], dt.float32, tag="tmp4")
                    nc.vector.tensor_tensor(
                        out=tmp4[:].rearrange("p (o c i) -> p o c i", o=16, c=3),
                        in0=wp[:, 768:1024]
                        .rearrange("p (o i) -> p o i", o=16)
                        .unsqueeze(2)
                        .broadcast_to([P, 16, 3, 16]),
                        in1=v_ci.unsqueeze(1).broadcast_to([P, 16, 3, 16]),
                        op=Alu.mult,
                    )
                    out4 = sb.tile([P, 48], dt.float32, tag="out4")
                    nc.vector.tensor_reduce(
                        out=out4[:],
                        in_=tmp4[:].rearrange("p (g i) -> p g i", i=16),
                        axis=mybir.AxisListType.X,
                        op=Alu.add,
                    )

                    feat = sb.tile([P, 64], dt.float32, tag="feat")
                    nc.vector.scalar_tensor_tensor(
                        out=feat[:, 0:16],
                        in0=M[:, 0:16],
                        scalar=shs,
                        in1=M[:, 16:32],
                        op0=Alu.mult,
                        op1=Alu.add,
                    )
                    tv = sb.tile([P, 48], dt.float32, tag="tv")
                    nc.vector.tensor_tensor(
                        out=tv[:].rearrange("p (o c) -> p o c", c=3),
                        in0=M[:, 32:48].unsqueeze(2).broadcast_to([P, 16, 3]),
                        in1=shv.unsqueeze(1).broadcast_to([P, 16, 3]),
                        op=Alu.mult,
                    )
                    nc.vector.scalar_tensor_tensor(
                        out=feat[:, 16:64],
                        in0=out4[:],
                        scalar=shs,
                        in1=tv[:],
                        op0=Alu.mult,
                        op1=Alu.add,
                    )

                    # dst one-hot selection matrix; scatter via PE accumulate
                    S = sb.tile([P, P], dt.float32, tag="S")
                    nc.vector.tensor_tensor(
                        out=S[:],
                        in0=dstf_sb[:, t : t + 1].to_broadcast([P, P]),
                        in1=iota_f[:],
                        op=Alu.is_equal,
                    )
                    nc.tensor.matmul(
                        out=acc[:], lhsT=S[:], rhs=feat[:],
                        start=(j == 0), stop=(j == t_b - 1),
                    )
                osb = sb.tile([P, 64], dt.float32, tag="osb")
                nc.scalar.activation(osb[:], acc[:], Act.Copy)
                nc.sync.dma_start(outp[b * BLK : (b + 1) * BLK, :], osb[:])
    nc.compile()
    return nc


def _prep(inputs):
    nf = np.ascontiguousarray(np.asarray(inputs["node_features"], dtype=np.float32))
    src = np.asarray(inputs["edge_src"]).astype(np.int64)
    dst = np.asarray(inputs["edge_dst"]).astype(np.int64)
    attr = np.asarray(inputs["edge_attr"], dtype=np.float32)
    sc = np.asarray(inputs["edge_scalars"], dtype=np.float32)
    w1 = np.asarray(inputs["fc_w1"], dtype=np.float32)
    w2 = np.asarray(inputs["fc_w2"], dtype=np.float32)

    fw1 = np.ascontiguousarray((w1 / np.sqrt(3.0)).astype(np.float32))
    # fc_w2 [256, (path,i,o)] -> [256, (path,o,i)], with all norms folded in
    w2r = w2.reshape(256, 4, MUL, MUL).transpose(0, 1, 3, 2).copy()
    scale = (
        (1.0 / np.sqrt(256.0))      # fc net layer 2
        * (1.0 / np.sqrt(2.0 * MUL))  # tensor-product path normalization
        * (1.0 / np.sqrt(16.0))     # NUM_NEIGHBORS normalization
    )
    w2r *= scale
    w2r[:, 1] *= 1.0 / np.sqrt(3.0)  # dot normalization (path 2 only)
    fw2 = np.ascontiguousarray(w2r.reshape(256, 1024).astype(np.float32))

    order = np.argsort(dst, kind="stable")
    srcs, dsts = src[order], dst[order]
    attrs, scs = attr[order], sc[order]

    core_of = dsts // NODES_PER_CORE
    local = dsts - core_of * NODES_PER_CORE
    blk = local // BLK
    gb = core_of * BLOCKS + blk
    counts = np.bincount(gb, minlength=N_CORES * BLOCKS)
    t_b = max(1, int(math.ceil(counts.max() / P)))
    n_tiles = BLOCKS * t_b
    e_pad = n_tiles * P

    seg_start = np.zeros(N_CORES * BLOCKS + 1, np.int64)
    np.cumsum(counts, out=seg_start[1:])

    in_maps = []
    for c in range(N_CORES):
        src_c = np.zeros(e_pad, np.int32)
        dst_c = np.full(e_pad, 1000.0, np.float32)  # out-of-window => no scatter
        attr_c = np.zeros((e_pad, 4), np.float32)
        sc_c = np.zeros((e_pad, 3), np.float32)
        for b in range(BLOCKS):
            g = c * BLOCKS + b
            a0, a1 = int(seg_start[g]), int(seg_start[g + 1])
            n = a1 - a0
            off = b * t_b * P
            src_c[off : off + n] = srcs[a0:a1]
            dst_c[off : off + n] = (local[a0:a1] - b * BLK).astype(np.float32)
            attr_c[off : off + n] = attrs[a0:a1]
            sc_c[off : off + n] = scs[a0:a1]
        in_maps.append(
            {
                "xg_tab": nf,
                "srcT": np.ascontiguousarray(src_c.reshape(n_tiles, P).T),
                "dstf": np.ascontiguousarray(dst_c.reshape(n_tiles, P).T),
                "attrP": np.ascontiguousarray(
                    attr_c.reshape(n_tiles, P, 4)
                    .transpose(1, 0, 2)
                    .reshape(P, 4 * n_tiles)
                ),
                "scT": np.ascontiguousarray(sc_c.T),
                "fw1": fw1,
                "fw2": fw2,
            }
        )
    return in_maps, n_tiles, t_b


def kernel(**inputs) -> np.ndarray:
    from concourse.bass_interp import get_hw_module
    from concourse.bass_utils import run_bass_kernel_spmd

    in_maps, n_tiles, t_b = _prep(inputs)
    key = (n_tiles, t_b)
    if key not in _CACHE:
        _CACHE[key] = _build(n_tiles, t_b)
    nc = _CACHE[key]
    old = nc.m
    nc.m = get_hw_module(nc.m)
    try:
        res = run_bass_kernel_spmd(nc, in_maps, core_ids=list(range(N_CORES)))
    finally:
        nc.m = old
    out = np.concatenate(
        [res.results[c]["outp"][:NODES_PER_CORE] for c in range(N_CORES)], axis=0
    )
    return np.ascontiguousarray(out.astype(np.float32))
